# revision 15
# baseline (speedup 1.0000x reference)
"""MLA q/kv projection kernel for Trainium2, 8 NeuronCores, SPMD data-parallel
over the token dimension (512 tokens per core).

Per-core pipeline (token-major layouts throughout):
  mm1:  c_full[512, 2112] = x_shard @ [wq_a | wkv]      (bf16 x bf16 -> fp32 PSUM)
  norm: cq = c_full[:, :1536] * rsqrt(mean(cq^2)+eps)   (gamma_cq folded into wq_b)
        ckv = rmsnorm(c_full[:, 1536:2048]) * gamma_ckv
        k_rope = rope(c_full[:, 2048:2112])
  tr:   cqnT = transpose(cq_n)  via PE-transpose        (stationary operand of mm2)
  mm2:  q[512, 24576] = cq_n @ wq_b_reordered
        rope on the per-head rope slices, strided DMA back to the original
        column order.

Host-side prep (free — not on the device critical path): shard+transpose
token_x, concat down-proj weights, fold gamma_cq into wq_b and group its
columns nope-first/rope-last, pre-repeat cos/sin 8x per head.
"""

import numpy as np

import concourse.bass as bass
import concourse.tile as tile
from concourse import mybir
from concourse.bass_utils import run_bass_kernel_spmd
from concourse.masks import make_identity
from concourse.vector_clock import ScopedClock, VectorClock

F32 = mybir.dt.float32
BF16 = mybir.dt.bfloat16

N_CORES = 8
T = 4096
TC = T // N_CORES           # 512 tokens per core
MT = TC // 128              # 4 token tiles
H = 7168
KH = H // 128               # 56 contraction tiles for mm1
L = 1536                    # q latent
KL = L // 128               # 12 contraction tiles for mm2
KV_RANK = 512
R = 64                      # rope dims
WDN = L + KV_RANK + R       # 2112 = fused down-proj width
N_HEADS = 128
QK_NOPE = 128
DN = N_HEADS * (QK_NOPE + R)   # 24576
NT2 = DN // 512                # 48 n-tiles for mm2
NOPE_TILES = N_HEADS * QK_NOPE // 512   # 32 (4 heads each)
ROPE_TILES = N_HEADS * R // 512         # 16 (8 heads each)
OUTW = DN + KV_RANK + R     # 25152
EPS = 1e-6


def split_multi_waits(nc, limit=1):
    """Walrus in this toolchain accepts at most one sync-wait command per
    TPB instruction. Hoist extra waits onto single-wait NoOps inserted
    immediately before the offending instruction on the same engine."""
    skip = (mybir.InstAllEngineBarrier, mybir.InstEventSemaphore)
    for f in nc.m.functions:
        for bb in f.blocks:
            new_insts = []
            changed = False
            for inst in bb.instructions:
                si = inst.sync_info
                waits = list(si.on_wait) if si is not None and si.on_wait else []
                if len(waits) > limit and not isinstance(inst, skip):
                    for w in waits[:-limit]:
                        nop = mybir.InstNoOp(
                            name=nc.get_next_instruction_name(),
                            sync_info=mybir.SyncInfo(on_wait=[w], on_update=[]),
                            bass_nofuse=True,
                            engine=inst.engine,
                        )
                        new_insts.append(nop)
                    inst.sync_info = mybir.SyncInfo(
                        on_wait=waits[-limit:], on_update=list(si.on_update))
                    changed = True
                new_insts.append(inst)
            if changed:
                bb.instructions = new_insts
    return nc


class PatchedTC(tile.TileContext):
    """Workaround for a walrus codegen limit in this toolchain: the kernel-tail
    SP Drain instruction only accepts ONE sync-wait command, while Tile attaches
    one wait per active processor. Chain single-wait drains instead."""

    def _drain_and_barrier(self, tick_clock, wait_clock):
        nc = self.nc
        gc = tick_clock.global_clock
        nprocs = len(gc)
        procs = [p for p in range(nprocs) if gc[p] > 0] or [0]
        for p in procs:
            d = nc.sync.drain()
            vc = VectorClock([0] * nprocs)
            vc.require_at_least(p, gc[p])
            wait_clock.add_sem_waits(d.ins, ScopedClock({None: vc}))
        nc.all_engine_barrier()
        assert self.sems is not None
        popped = nc._tile_sem_poison_stack.pop()
        assert popped is self._sem_poison
        nc.clear_and_free_semaphores(list(self.sems.allocated().values()))
        nc.all_engine_barrier()


def build_nc(split=True):
    import os
    phases = os.environ.get("MLA_PHASES", "123")
    nc = bass.Bass()
    xt = nc.dram_tensor("xt", [H, TC], BF16, kind="ExternalInput")
    wd = nc.dram_tensor("wd", [H, WDN], BF16, kind="ExternalInput")
    wb = nc.dram_tensor("wb", [L, DN], BF16, kind="ExternalInput")
    cs = nc.dram_tensor("cs", [TC, 1024], F32, kind="ExternalInput")
    gkv = nc.dram_tensor("gkv", [KV_RANK], F32, kind="ExternalInput")
    out = nc.dram_tensor("out", [TC, OUTW], F32, kind="ExternalOutput")

    out_ap = out.ap()
    qv = out_ap[:, 0:DN].rearrange("t (h d) -> t h d", d=QK_NOPE + R)

    with PatchedTC(nc) as tc:
        with (
            tc.tile_pool(name="consts", bufs=1) as p_const,
            tc.tile_pool(name="cs", bufs=1) as p_cs,
            tc.tile_pool(name="cqnT", bufs=1) as p_cqnT,
            tc.tile_pool(name="xt", bufs=1) as p_xt,
            tc.tile_pool(name="wd", bufs=6) as p_wd,
            tc.tile_pool(name="cfull", bufs=1) as p_c,
            tc.tile_pool(name="scr", bufs=2) as p_scr,
            tc.tile_pool(name="stats", bufs=1) as p_stats,
            tc.tile_pool(name="tmp", bufs=4) as p_tmp,
            tc.tile_pool(name="wb", bufs=8) as p_wb,
            tc.tile_pool(name="qout", bufs=4) as p_qout,
            tc.tile_pool(name="psum", bufs=8, space="PSUM") as p_ps,
        ):
            # ---- constants ----
            ident = p_const.tile([128, 128], BF16, tag="ident", name="ident")
            make_identity(nc, ident)
            eps_t = p_const.tile([128, 1], F32, tag="eps", name="eps_t")
            nc.vector.memset(eps_t, EPS)
            gamma_b = p_const.tile([128, KV_RANK], F32, tag="gamma", name="gamma_b")
            g_ap = gkv.ap()
            nc.sync.dma_start(
                out=gamma_b,
                in_=bass.AP(tensor=g_ap.tensor, offset=g_ap.offset,
                            ap=[[0, 128]] + [list(p) for p in g_ap.ap]),
            )
            cs_sb = []
            for m in range(MT):
                t = p_cs.tile([128, 1024], F32, tag=f"cs{m}", name=f"cs{m}")
                nc.sync.dma_start(out=t, in_=cs.ap()[m * 128:(m + 1) * 128, :])
                cs_sb.append(t)

            cqnT = p_cqnT.tile([128, KL, TC], BF16, tag="cqnT", name="cqnT")

            # ---- mm1: c_full = x @ [wq_a | wkv] ----
            c_sb = [p_c.tile([128, WDN], F32, tag=f"c{m}", name=f"c{m}") for m in range(MT)]
            xt_tiles = {}
            n_cols = [512, 512, 512, 512, 64]
            for n in range(5):
                w = n_cols[n]
                ps_t = [p_ps.tile([128, 512], F32, tag="ps", name="ps") for _ in range(MT)]
                for k in range(KH):
                    if n == 0:
                        xt_tiles[k] = p_xt.tile([128, TC], BF16, tag=f"xt{k}", name=f"xt{k}")
                        nc.sync.dma_start(
                            out=xt_tiles[k],
                            in_=xt.ap()[k * 128:(k + 1) * 128, :])
                    wd_t = p_wd.tile([128, 512], BF16, tag="wd", name="wd_t")
                    nc.sync.dma_start(
                        out=wd_t[:, 0:w],
                        in_=wd.ap()[k * 128:(k + 1) * 128, n * 512:n * 512 + w])
                    for m in range(MT):
                        nc.tensor.matmul(
                            ps_t[m][:, 0:w],
                            lhsT=xt_tiles[k][:, m * 128:(m + 1) * 128],
                            rhs=wd_t[:, 0:w],
                            start=(k == 0), stop=(k == KH - 1))
                for m in range(MT):
                    nc.vector.tensor_copy(
                        out=c_sb[m][:, n * 512:n * 512 + w],
                        in_=ps_t[m][:, 0:w])

            # ---- q norm: cq_n = cq * rsqrt(mean(cq^2) + eps), cast bf16 ----
            cqn = []
            for m in range(MT) if "2" in phases else []:
                scr = p_scr.tile([128, L], BF16, tag="scr", name="scr")
                st = p_stats.tile([128, 1], F32, tag=f"stq{m}", name=f"stq{m}")
                nc.scalar.activation(
                    out=scr, in_=c_sb[m][:, 0:L],
                    func=mybir.ActivationFunctionType.Square,
                    accum_out=st)
                nc.scalar.activation(
                    out=st, in_=st,
                    func=mybir.ActivationFunctionType.Sqrt,
                    bias=eps_t, scale=1.0 / L)
                nc.vector.reciprocal(out=st, in_=st)
                cq_m = p_c.tile([128, L], BF16, tag=f"cqn{m}", name=f"cqn{m}")
                nc.scalar.mul(out=cq_m, in_=c_sb[m][:, 0:L], mul=st)
                cqn.append(cq_m)

            # ---- transpose cq_n -> cqnT (stationary operand of mm2) ----
            for m in range(len(cqn)):
                for j in range(KL):
                    tp = p_ps.tile([128, 128], BF16, tag="ps", name="tps")
                    nc.tensor.transpose(
                        tp,
                        cqn[m][:, j * 128:(j + 1) * 128],
                        ident)
                    nc.vector.tensor_copy(
                        out=cqnT[:, j, m * 128:(m + 1) * 128],
                        in_=tp)

            # ---- kv path: ckv rmsnorm * gamma, k_rope rope, store ----
            for m in range(MT) if "2" in phases else []:
                kv_sb = p_c.tile([128, KV_RANK + R], F32, tag=f"kv{m}", name=f"kv{m}")
                scr = p_scr.tile([128, L], BF16, tag="scr", name="scr")
                st = p_stats.tile([128, 1], F32, tag=f"stk{m}", name=f"stk{m}")
                nc.scalar.activation(
                    out=scr[:, 0:KV_RANK], in_=c_sb[m][:, L:L + KV_RANK],
                    func=mybir.ActivationFunctionType.Square,
                    accum_out=st)
                nc.scalar.activation(
                    out=st, in_=st,
                    func=mybir.ActivationFunctionType.Sqrt,
                    bias=eps_t, scale=1.0 / KV_RANK)
                nc.vector.reciprocal(out=st, in_=st)
                nc.vector.tensor_scalar_mul(
                    out=kv_sb[:, 0:KV_RANK],
                    in0=c_sb[m][:, L:L + KV_RANK],
                    scalar1=st)
                nc.vector.tensor_mul(
                    out=kv_sb[:, 0:KV_RANK],
                    in0=kv_sb[:, 0:KV_RANK],
                    in1=gamma_b)
                # k_rope: x1=c[...:+32], x2=c[+32:+64]
                x1 = c_sb[m][:, L + KV_RANK:L + KV_RANK + 32]
                x2 = c_sb[m][:, L + KV_RANK + 32:L + KV_RANK + 64]
                cos1 = cs_sb[m][:, 0:32]
                cos2 = cs_sb[m][:, 32:64]
                sin1 = cs_sb[m][:, 512:544]
                sin2 = cs_sb[m][:, 544:576]
                ta = p_tmp.tile([128, 256], F32, tag="ta", name="ta")
                tb = p_tmp.tile([128, 256], F32, tag="tb", name="tb")
                nc.vector.tensor_mul(out=ta[:, 0:32], in0=x1, in1=cos1)
                nc.vector.tensor_mul(out=tb[:, 0:32], in0=x2, in1=sin1)
                nc.vector.tensor_sub(
                    out=kv_sb[:, KV_RANK:KV_RANK + 32],
                    in0=ta[:, 0:32], in1=tb[:, 0:32])
                ta2 = p_tmp.tile([128, 256], F32, tag="ta", name="ta")
                tb2 = p_tmp.tile([128, 256], F32, tag="tb", name="tb")
                nc.vector.tensor_mul(out=ta2[:, 0:32], in0=x2, in1=cos2)
                nc.vector.tensor_mul(out=tb2[:, 0:32], in0=x1, in1=sin2)
                nc.vector.tensor_add(
                    out=kv_sb[:, KV_RANK + 32:KV_RANK + 64],
                    in0=ta2[:, 0:32], in1=tb2[:, 0:32])
                nc.sync.dma_start(
                    out=out_ap[m * 128:(m + 1) * 128, DN:OUTW],
                    in_=kv_sb)

            # ---- mm2: q = cq_n @ wb (reordered), rope, store ----
            for n in range(NT2) if "3" in phases else []:
                ps_t = [p_ps.tile([128, 512], F32, tag="ps", name="ps") for _ in range(MT)]
                for k in range(KL):
                    wb_t = p_wb.tile([128, 512], BF16, tag="wb", name="wb_t")
                    nc.sync.dma_start(
                        out=wb_t,
                        in_=wb.ap()[k * 128:(k + 1) * 128, n * 512:(n + 1) * 512])
                    for m in range(MT):
                        nc.tensor.matmul(
                            ps_t[m],
                            lhsT=cqnT[:, k, m * 128:(m + 1) * 128],
                            rhs=wb_t,
                            start=(k == 0), stop=(k == KL - 1))
                if n < NOPE_TILES:
                    for m in range(MT):
                        q_sb = p_qout.tile([128, 512], F32, tag="q", name="q_sb")
                        nc.vector.tensor_copy(out=q_sb, in_=ps_t[m])
                        nc.sync.dma_start(
                            out=qv[m * 128:(m + 1) * 128,
                                   4 * n:4 * n + 4, 0:QK_NOPE],
                            in_=q_sb.rearrange("p (h d) -> p h d", d=QK_NOPE))
                else:
                    i = n - NOPE_TILES
                    for m in range(MT):
                        q_sb = p_qout.tile([128, 512], F32, tag="q", name="q_sb")
                        x = ps_t[m].rearrange("p (h d) -> p h d", d=R)
                        qo = q_sb.rearrange("p (h d) -> p h d", d=R)
                        cosr = cs_sb[m][:, 0:512].rearrange(
                            "p (h d) -> p h d", d=R)
                        sinr = cs_sb[m][:, 512:1024].rearrange(
                            "p (h d) -> p h d", d=R)
                        ta = p_tmp.tile([128, 256], F32, tag="ta", name="ta")
                        tb = p_tmp.tile([128, 256], F32, tag="tb", name="tb")
                        va = ta.rearrange("p (h d) -> p h d", d=32)
                        vb = tb.rearrange("p (h d) -> p h d", d=32)
                        nc.vector.tensor_mul(
                            out=va, in0=x[:, :, 0:32], in1=cosr[:, :, 0:32])
                        nc.vector.tensor_mul(
                            out=vb, in0=x[:, :, 32:64], in1=sinr[:, :, 0:32])
                        nc.vector.tensor_sub(
                            out=qo[:, :, 0:32], in0=va, in1=vb)
                        ta2 = p_tmp.tile([128, 256], F32, tag="ta", name="ta")
                        tb2 = p_tmp.tile([128, 256], F32, tag="tb", name="tb")
                        va2 = ta2.rearrange("p (h d) -> p h d", d=32)
                        vb2 = tb2.rearrange("p (h d) -> p h d", d=32)
                        nc.vector.tensor_mul(
                            out=va2, in0=x[:, :, 32:64], in1=cosr[:, :, 32:64])
                        nc.vector.tensor_mul(
                            out=vb2, in0=x[:, :, 0:32], in1=sinr[:, :, 32:64])
                        nc.vector.tensor_add(
                            out=qo[:, :, 32:64], in0=va2, in1=vb2)
                        nc.sync.dma_start(
                            out=qv[m * 128:(m + 1) * 128,
                                   8 * i:8 * i + 8, QK_NOPE:QK_NOPE + R],
                            in_=qo)
    if split:
        split_multi_waits(nc)
    return nc


def prep_inputs(token_x, wq_a, wq_b, wkv, rope_cos, rope_sin, gamma_cq,
                gamma_ckv):
    """Host-side sharding + layout prep. Returns in_maps for the 8 cores."""
    bf16 = mybir.dt.np(BF16)
    wd = np.concatenate([wq_a, wkv], axis=1).astype(bf16)          # [H, 2112]
    wbs = (wq_b.astype(np.float32)
           * gamma_cq.astype(np.float32)[:, None]).reshape(L, N_HEADS,
                                                           QK_NOPE + R)
    wb = np.concatenate(
        [wbs[:, :, :QK_NOPE].reshape(L, -1), wbs[:, :, QK_NOPE:].reshape(L, -1)],
        axis=1).astype(bf16)                                       # [L, 24576]
    gkv = np.ascontiguousarray(gamma_ckv.astype(np.float32))
    in_maps = []
    for c in range(N_CORES):
        sl = slice(c * TC, (c + 1) * TC)
        xt = np.ascontiguousarray(token_x[sl].T).astype(bf16)      # [H, TC]
        cos_rep = np.tile(rope_cos[sl].astype(np.float32), (1, 8)) # [TC, 512]
        sin_rep = np.tile(rope_sin[sl].astype(np.float32), (1, 8))
        cs = np.ascontiguousarray(
            np.concatenate([cos_rep, sin_rep], axis=1))            # [TC, 1024]
        in_maps.append({"xt": xt, "wd": wd, "wb": wb, "cs": cs, "gkv": gkv})
    return in_maps


def kernel(token_x, wq_a, wq_b, wkv, rope_cos, rope_sin, gamma_cq, gamma_ckv):
    token_x, wq_a, wq_b, wkv, rope_cos, rope_sin, gamma_cq, gamma_ckv = (
        np.asarray(a) for a in (token_x, wq_a, wq_b, wkv, rope_cos, rope_sin,
                                gamma_cq, gamma_ckv))
    in_maps = prep_inputs(token_x, wq_a, wq_b, wkv, rope_cos, rope_sin,
                          gamma_cq, gamma_ckv)
    nc = build_nc()
    res = run_bass_kernel_spmd(nc, in_maps, list(range(N_CORES)))
    return np.concatenate([res.results[c]["out"] for c in range(N_CORES)],
                          axis=0)
